# revision 6
# baseline (speedup 1.0000x reference)
"""Trainium2 Bass kernel for nn_CachedCompressedLinear.

out[16, 11008] = x[16, 4096] @ ((w_q - 128) * scale).T + bias

Sharding: column-parallel over 8 NeuronCores; each core owns a 1376-wide
slice of out_features (8 * 1376 = 11008).

v2: the int32 weight codes are packed to uint8 on the host (values are
0..255, so the upper 3 bytes in HBM are zeros) cutting weight DMA 4x to
5.64 MB/core.  On-device decode uint8 -> bf16 (with the -128 shift fused)
is split between DVE (cols 0:864 of each k-tile, 2x perf mode) and ACT
(cols 864:1376, 1x) so it hides under the matmuls.  Matmuls run
back-to-back (progressive DMA group sizes fill the pipeline early) so the
PE stays at its warm 2.4 GHz clock; weights stream as bf16 moving data in
three PSUM chunks (512, 352, 512) per k-tile.  x is replicated,
pre-transposed and split into bf16 hi/lo halves so the bf16 matmul
carries fp32-level precision.  The per-tensor scale and the bias are
applied on the small f32 output in the epilogue (bias via K=1 matmuls
folded into the last k-tile's accumulation).
"""

import sys

if "/opt/trn_rl_repo" not in sys.path:
    sys.path.insert(0, "/opt/trn_rl_repo")

import numpy as np
import ml_dtypes

IN_F = 4096
OUT_F = 11008
BATCH = 16
N_CORES = 8
O_PER = 1376  # out_features per core
K_TILES = IN_F // 128  # 32
M = 48  # stationary columns: x_hi [0:16] | zeros [16:32] | x_lo [32:48]
LO = 32
# (offset, width, engine): DVE decodes [0, 864), ACT decodes [864, 1376)
CHUNKS = [(0, 512, "dve"), (512, 352, "dve"), (864, 512, "act")]
DVE_W = 864
ACT_W = 512
# k-tile group sizes: small groups at both ends — the start fills the
# DMA->decode->MM pipeline quickly, the end drains it without a coarse
# serialized last group
GROUPS = [1, 1, 2, 4, 8, 8, 4, 2, 1, 1]

_BUILT = None


def _build():
    """Build the (SPMD, per-core) Bass program once."""
    import concourse.bass as bass
    import concourse.tile as tile
    from concourse import bacc, mybir

    dt = mybir.dt
    nc = bacc.Bacc("TRN2", target_bir_lowering=False, debug=False)

    wt8 = nc.dram_tensor("wt8", [128, K_TILES * O_PER], dt.uint8,
                         kind="ExternalInput")
    xt2 = nc.dram_tensor(
        "xt2", [128, (K_TILES + 1) * M], dt.bfloat16, kind="ExternalInput"
    )
    bias_rep = nc.dram_tensor(
        "bias_rep", [1, O_PER], dt.float32, kind="ExternalInput"
    )
    s_col = nc.dram_tensor("s_col", [BATCH, 1], dt.float32, kind="ExternalInput")
    out = nc.dram_tensor("out", [BATCH, O_PER], dt.float32, kind="ExternalOutput")

    BIASBLK = K_TILES  # extra stationary block holding the bias one-hot
    with tile.TileContext(nc) as tc:
        with (
            tc.tile_pool(name="consts", bufs=1) as consts,
            tc.tile_pool(name="w8", bufs=3) as w8p,
            tc.tile_pool(name="wbA", bufs=3) as wbAp,
            tc.tile_pool(name="wbB", bufs=3) as wbBp,
            tc.tile_pool(name="psum", bufs=1, space=bass.MemorySpace.PSUM) as psump,
            tc.tile_pool(name="outp", bufs=1) as outp,
        ):
            # x (hi|lo) host-prepacked in SBUF layout + bias one-hot block.
            # x rides the fast SWDGE ring ahead of the weights (it gates the
            # first LDWEIGHTS); bias/s go on the sync HWDGE ring (needed
            # only at the last k-tile / epilogue).
            x_sb = consts.tile([128, (K_TILES + 1) * M], dt.bfloat16)
            nc.gpsimd.dma_start(x_sb[:], xt2[:])
            bias_sb = consts.tile([1, O_PER], dt.float32)
            nc.sync.dma_start(bias_sb[:], bias_rep[:])
            s_sb = consts.tile([BATCH, 1], dt.float32)
            nc.sync.dma_start(s_sb[:], s_col[:])

            psums = [
                psump.tile([M, w], dt.float32, name=f"ps{i}", tag=f"ps{i}")
                for i, (_, w, _e) in enumerate(CHUNKS)
            ]

            def epilogue(i, o, w):
                # hi -> ACT (Copy, scale fused); lo -> DVE (mul by s);
                # sum -> DVE; per-chunk output DMA.
                his = outp.tile([BATCH, w], dt.float32, name=f"his{i}")
                nc.scalar.activation(
                    his[:],
                    psums[i][0:BATCH, :],
                    mybir.ActivationFunctionType.Copy,
                    scale=s_sb[:, 0:1],
                )
                los = outp.tile([BATCH, w], dt.float32, name=f"los{i}")
                nc.vector.tensor_scalar_mul(
                    los[:], psums[i][LO:LO + BATCH, :], s_sb[:, 0:1]
                )
                comb = outp.tile([BATCH, w], dt.float32, name=f"comb{i}")
                nc.vector.tensor_add(comb[:], his[:], los[:])
                nc.sync.dma_start(out[:][:, o:o + w], comb[:])

            bq = {}

            GMAX = max(GROUPS)
            k0 = 0
            for gi, G in enumerate(GROUPS):
                wt_t = w8p.tile([128, GMAX, O_PER], dt.uint8, tag="w8")
                nc.gpsimd.dma_start(
                    wt_t[:, 0:G, :],
                    wt8[:, k0 * O_PER:(k0 + G) * O_PER],
                )
                # decode: DVE takes cols [0, 864), ACT takes [864, 1376)
                wbA = wbAp.tile([128, GMAX, DVE_W], dt.bfloat16, tag="wA")
                nc.vector.tensor_scalar_add(
                    wbA[:, 0:G, :], wt_t[:, 0:G, 0:DVE_W], -128.0
                )
                wbB = wbBp.tile([128, GMAX, ACT_W], dt.bfloat16, tag="wB")
                nc.scalar.activation(
                    wbB[:, 0:G, :], wt_t[:, 0:G, DVE_W:O_PER],
                    mybir.ActivationFunctionType.Copy, bias=-128.0
                )
                for t in range(G):
                    k = k0 + t
                    last = k == K_TILES - 1

                    def mv_of(i):
                        o, w, eng = CHUNKS[i]
                        if eng == "dve":
                            return wbA[:, t, o:o + w]
                        return wbB[:, t, o - DVE_W:o - DVE_W + w]

                    if not last:
                        for i in range(len(CHUNKS)):
                            nc.tensor.matmul(
                                psums[i][:, :],
                                x_sb[:, k * M:(k + 1) * M],
                                mv_of(i),
                                start=(k == 0),
                                stop=False,
                            )
                    else:
                        # close chunk-by-chunk: bias K=1 matmuls + final
                        # MM per chunk, epilogue of chunk i overlaps the
                        # closing matmuls of chunk i+1
                        for i, (o, w, _e) in enumerate(CHUNKS):
                            for bvec in (bq["h"], bq["l"]):
                                nc.tensor.matmul(
                                    psums[i][:, :],
                                    x_sb[0:1, BIASBLK * M:(BIASBLK + 1) * M],
                                    bvec[0:1, o:o + w],
                                    start=False,
                                    stop=False,
                                )
                            nc.tensor.matmul(
                                psums[i][:, :],
                                x_sb[:, k * M:(k + 1) * M],
                                mv_of(i),
                                start=False,
                                stop=True,
                            )
                            epilogue(i, o, w)
                k0 += G

                if gi == 1:
                    # bias/s in bf16 hi/lo, fed to PSUM via two K=1 matmuls
                    # in the last k-tile so the epilogue needs no separate
                    # bias add.  Emitted after the first two decode groups
                    # so the small DVE prep ops don't delay the first MMs.
                    rs = consts.tile([1, 1], dt.float32)
                    nc.vector.reciprocal(rs[:], s_sb[0:1, 0:1])
                    bq32 = consts.tile([1, O_PER], dt.float32)
                    nc.vector.tensor_scalar_mul(
                        bq32[:], bias_sb[0:1, :], rs[0:1, 0:1])
                    bqh = consts.tile([1, O_PER], dt.bfloat16)
                    nc.vector.tensor_copy(bqh[:], bq32[:])
                    bql32 = consts.tile([1, O_PER], dt.float32)
                    nc.vector.tensor_sub(bql32[:], bq32[:], bqh[:])
                    bql = consts.tile([1, O_PER], dt.bfloat16)
                    nc.vector.tensor_copy(bql[:], bql32[:])
                    bq["h"], bq["l"] = bqh, bql

    nc.compile()
    return nc


def _get_built():
    global _BUILT
    if _BUILT is None:
        _BUILT = _build()
    return _BUILT


def make_in_maps(x, w_q, scale, bias):
    """Host-side shard + layout prep. Returns per-core input dicts."""
    x = np.asarray(x, dtype=np.float32)
    w_q = np.asarray(w_q, dtype=np.int32)
    scale = np.asarray(scale, dtype=np.float32)
    bias = np.asarray(bias, dtype=np.float32)

    xT = np.ascontiguousarray(x.T)  # [4096, 16]
    xh = xT.astype(ml_dtypes.bfloat16)
    xl = (xT - xh.astype(np.float32)).astype(ml_dtypes.bfloat16)
    x48 = np.zeros((IN_F, M), dtype=ml_dtypes.bfloat16)  # [4096, 48]
    x48[:, :BATCH] = xh
    x48[:, LO:LO + BATCH] = xl
    # prepack to the SBUF layout [128, K_TILES*M]: partition p holds,
    # for each k-tile t, the stationary block row (t*128 + p)
    xt2 = np.zeros((128, (K_TILES + 1) * M), dtype=ml_dtypes.bfloat16)
    xt2[:, :K_TILES * M] = (
        x48.reshape(K_TILES, 128, M).transpose(1, 0, 2).reshape(128, K_TILES * M)
    )
    # bias one-hot block: partition 0, first BATCH stationary columns = 1
    xt2[0, K_TILES * M:K_TILES * M + BATCH] = 1.0

    s_col = np.full((BATCH, 1), scale.reshape(-1)[0], dtype=np.float32)

    in_maps = []
    for c in range(N_CORES):
        # uint8 codes, transposed to [4096, 1376] then packed so partition
        # p holds, for k-tile t, row (t*128 + p): [128, 32*1376]
        wt_c = w_q[c * O_PER:(c + 1) * O_PER].T.astype(np.uint8)
        wt8_c = np.ascontiguousarray(
            wt_c.reshape(K_TILES, 128, O_PER)
            .transpose(1, 0, 2)
            .reshape(128, K_TILES * O_PER)
        )
        bias_c = np.ascontiguousarray(
            bias[c * O_PER:(c + 1) * O_PER].reshape(1, O_PER)
        )
        in_maps.append(
            {"wt8": wt8_c, "xt2": xt2, "bias_rep": bias_c, "s_col": s_col}
        )
    return in_maps


def run(inputs, trace=False):
    """Run on the 8 NeuronCores. Returns (full_output, BassKernelResults)."""
    from concourse.bass_utils import run_bass_kernel_spmd

    in_maps = make_in_maps(**inputs)
    nc = _get_built()
    res = run_bass_kernel_spmd(nc, in_maps, list(range(N_CORES)), trace=trace)
    parts = [np.asarray(res.results[c]["out"]) for c in range(N_CORES)]
    full = np.concatenate(parts, axis=1)[:, :OUT_F].astype(np.float32)
    return full, res


def kernel(**inputs) -> np.ndarray:
    full, _ = run(inputs, trace=False)
    return full


# revision 9
# speedup vs baseline: 1.0006x; 1.0006x over previous
"""Trainium2 Bass kernel for nn_CachedCompressedLinear.

out[16, 11008] = x[16, 4096] @ ((w_q - 128) * scale).T + bias

Sharding: column-parallel over 8 NeuronCores; each core owns a 1376-wide
slice of out_features (8 * 1376 = 11008).

The int32 weight codes are packed to uint8 on the host (values are 0..255,
so the upper 3 bytes in HBM are zeros), cutting weight DMA 4x to
5.64 MB/core.  The weight stream is split over three DMA rings — gpsimd
SWDGE (k-tiles 0..19, fast ring, consumed first), scalar HWDGE (20..25)
and sync HWDGE (26..31) — so the slow HWDGE rings carry tiles consumed
late while the aggregate approaches the HBM ceiling.  On-device decode
uint8 -> bf16 computes (c - 128) * s directly (scale fused into the
decode, so the epilogue is a single psum_hi + psum_lo add per chunk) and
is split between DVE (cols 0:864 of each k-tile, 2x perf mode) and ACT
(cols 864:1376).  Dummy matmuls on a memset tile warm the PE's HAM clock
gate to 2.4 GHz before the first real matmul; progressive group sizes at
both ends keep the DMA->decode->MM pipeline full.  x is replicated,
pre-transposed and split into bf16 hi/lo halves for fp32-level precision;
bias is host-split into bf16 hi/lo rows and folded into the PSUM
accumulation by one K=2 matmul per chunk against a two-row one-hot block.
"""

import sys

if "/opt/trn_rl_repo" not in sys.path:
    sys.path.insert(0, "/opt/trn_rl_repo")

import numpy as np
import ml_dtypes

IN_F = 4096
OUT_F = 11008
BATCH = 16
N_CORES = 8
O_PER = 1376  # out_features per core
K_TILES = IN_F // 128  # 32
M = 48  # stationary columns: x_hi [0:16] | zeros [16:32] | x_lo [32:48]
LO = 32
# (offset, width, engine): DVE decodes [0, 864), ACT decodes [864, 1376)
CHUNKS = [(0, 512, "dve"), (512, 352, "dve"), (864, 512, "act")]
DVE_W = 864
ACT_W = 512
# (n_k_tiles, ring): rings gp/sc/sy run concurrently; slow HWDGE rings
# carry the k-tiles consumed last
GROUPS = [(1, "gp"), (1, "gp"), (2, "gp"), (4, "gp"), (6, "gp"), (6, "gp"),
          (6, "sc"), (6, "sy")]
N_WARM = 14  # dummy matmuls to warm the PE clock gate

_BUILT = None


def _build():
    """Build the (SPMD, per-core) Bass program once."""
    import concourse.bass as bass
    import concourse.tile as tile
    from concourse import bacc, mybir

    dt = mybir.dt
    alu = mybir.AluOpType
    nc = bacc.Bacc("TRN2", target_bir_lowering=False, debug=False)

    wt8 = nc.dram_tensor("wt8", [128, K_TILES * O_PER], dt.uint8,
                         kind="ExternalInput")
    xt2 = nc.dram_tensor(
        "xt2", [128, (K_TILES + 1) * M], dt.bfloat16, kind="ExternalInput"
    )
    bias_hl = nc.dram_tensor(
        "bias_hl", [2, O_PER], dt.bfloat16, kind="ExternalInput"
    )
    # col 0: s (replicated), col 1: -128*s
    sc2 = nc.dram_tensor("sc2", [128, 2], dt.float32, kind="ExternalInput")
    out = nc.dram_tensor("out", [BATCH, O_PER], dt.float32, kind="ExternalOutput")

    BIASBLK = K_TILES  # extra stationary block holding the bias one-hot
    with tile.TileContext(nc) as tc:
        with (
            tc.tile_pool(name="consts", bufs=1) as consts,
            tc.tile_pool(name="w8", bufs=3) as w8p,
            tc.tile_pool(name="wbA", bufs=3) as wbAp,
            tc.tile_pool(name="wbB", bufs=3) as wbBp,
            tc.tile_pool(name="psum", bufs=1, space=bass.MemorySpace.PSUM) as psump,
            tc.tile_pool(name="outp", bufs=1) as outp,
        ):
            rings = {"gp": nc.gpsimd, "sc": nc.scalar, "sy": nc.sync}

            # sync ring first sends the (tiny) scale, which gates every
            # decode op; gpsimd sends x (gates the first matmul) ahead of
            # its weight groups.
            sc_sb = consts.tile([128, 2], dt.float32)
            nc.sync.dma_start(sc_sb[:], sc2[:])
            x_sb = consts.tile([128, (K_TILES + 1) * M], dt.bfloat16)
            nc.gpsimd.dma_start(x_sb[:], xt2[:])

            # weight DMAs: issue all rings' transfers up-front in program
            # order; each ring is FIFO so within a ring the k-order holds.
            GMAX = max(g for g, _r in GROUPS)
            w_tiles = []
            k0 = 0
            for gi, (G, ring) in enumerate(GROUPS):
                if ring == "gp":
                    wt_t = w8p.tile([128, GMAX, O_PER], dt.uint8, tag="w8")
                else:
                    # early-issued, late-consumed: own allocation so the
                    # transfer is not serialized behind ring-buffer reuse
                    wt_t = w8p.tile([128, GMAX, O_PER], dt.uint8,
                                    tag=f"w8_{ring}")
                rings[ring].dma_start(
                    wt_t[:, 0:G, :],
                    wt8[:, k0 * O_PER:(k0 + G) * O_PER],
                )
                w_tiles.append((k0, G, wt_t))
                k0 += G

            # bias hi/lo rows (bf16, host-precomputed), needed at k=31
            bias_sb = consts.tile([2, O_PER], dt.bfloat16)
            nc.sync.dma_start(bias_sb[:], bias_hl[:])

            # warm the PE clock gate with dummy matmuls on a memset tile
            warm = consts.tile([128, 512], dt.bfloat16, name="warm")
            nc.vector.memset(warm[:], 1.0)
            ps_warm = psump.tile([M, 512], dt.float32, name="psw", tag="psw")
            for _ in range(N_WARM):
                nc.tensor.matmul(ps_warm[:], warm[:, 0:M], warm[:],
                                 start=True, stop=True)

            psums = [
                psump.tile([M, w], dt.float32, name=f"ps{i}", tag=f"ps{i}")
                for i, (_, w, _e) in enumerate(CHUNKS)
            ]

            out_rings = [nc.sync, nc.scalar, nc.gpsimd]

            def epilogue(i, o, w):
                # TT cannot read two PSUM operands: ACT copies hi to SBUF,
                # DVE adds the lo PSUM rows
                his = outp.tile([BATCH, w], dt.float32, name=f"his{i}")
                nc.scalar.copy(his[:], psums[i][0:BATCH, :])
                comb = outp.tile([BATCH, w], dt.float32, name=f"comb{i}")
                nc.vector.tensor_add(
                    comb[:], his[:], psums[i][LO:LO + BATCH, :]
                )
                out_rings[i].dma_start(out[:][:, o:o + w], comb[:])

            for k0, G, wt_t in w_tiles:
                # decode (c-128)*s: DVE cols [0,864), ACT cols [864,1376)
                wbA = wbAp.tile([128, GMAX, DVE_W], dt.bfloat16, tag="wA")
                nc.vector.tensor_scalar(
                    wbA[:, 0:G, :], wt_t[:, 0:G, 0:DVE_W],
                    -128.0, sc_sb[:, 0:1], alu.add, alu.mult,
                )
                wbB = wbBp.tile([128, GMAX, ACT_W], dt.bfloat16, tag="wB")
                nc.scalar.activation(
                    wbB[:, 0:G, :], wt_t[:, 0:G, DVE_W:O_PER],
                    mybir.ActivationFunctionType.Identity,
                    bias=sc_sb[:, 1:2], scale=sc_sb[:, 0:1],
                )
                for t in range(G):
                    k = k0 + t
                    last = k == K_TILES - 1

                    def mv_of(i):
                        o, w, eng = CHUNKS[i]
                        if eng == "dve":
                            return wbA[:, t, o:o + w]
                        return wbB[:, t, o - DVE_W:o - DVE_W + w]

                    if not last:
                        for i in range(len(CHUNKS)):
                            nc.tensor.matmul(
                                psums[i][:, :],
                                x_sb[:, k * M:(k + 1) * M],
                                mv_of(i),
                                start=(k == 0),
                                stop=False,
                            )
                    else:
                        # close chunk-by-chunk: K=2 bias matmul + final MM
                        # per chunk; epilogue of chunk i overlaps the
                        # closing matmuls of chunk i+1
                        for i, (o, w, _e) in enumerate(CHUNKS):
                            nc.tensor.matmul(
                                psums[i][:, :],
                                x_sb[0:2, BIASBLK * M:(BIASBLK + 1) * M],
                                bias_sb[0:2, o:o + w],
                                start=False,
                                stop=False,
                            )
                            nc.tensor.matmul(
                                psums[i][:, :],
                                x_sb[:, k * M:(k + 1) * M],
                                mv_of(i),
                                start=False,
                                stop=True,
                            )
                            epilogue(i, o, w)

    nc.compile()
    return nc


def _get_built():
    global _BUILT
    if _BUILT is None:
        _BUILT = _build()
    return _BUILT


def make_in_maps(x, w_q, scale, bias):
    """Host-side shard + layout prep. Returns per-core input dicts."""
    x = np.asarray(x, dtype=np.float32)
    w_q = np.asarray(w_q, dtype=np.int32)
    scale = np.asarray(scale, dtype=np.float32)
    bias = np.asarray(bias, dtype=np.float32)
    s = float(scale.reshape(-1)[0])

    xT = np.ascontiguousarray(x.T)  # [4096, 16]
    xh = xT.astype(ml_dtypes.bfloat16)
    xl = (xT - xh.astype(np.float32)).astype(ml_dtypes.bfloat16)
    x48 = np.zeros((IN_F, M), dtype=ml_dtypes.bfloat16)  # [4096, 48]
    x48[:, :BATCH] = xh
    x48[:, LO:LO + BATCH] = xl
    # prepack to the SBUF layout [128, K_TILES*M]: partition p holds,
    # for each k-tile t, the stationary block row (t*128 + p)
    xt2 = np.zeros((128, (K_TILES + 1) * M), dtype=ml_dtypes.bfloat16)
    xt2[:, :K_TILES * M] = (
        x48.reshape(K_TILES, 128, M).transpose(1, 0, 2).reshape(128, K_TILES * M)
    )
    # bias one-hot block: partitions 0 and 1, first BATCH stationary
    # columns = 1 (K=2 matmul adds bias_hi + bias_lo into the hi rows)
    xt2[0, K_TILES * M:K_TILES * M + BATCH] = 1.0
    xt2[1, K_TILES * M:K_TILES * M + BATCH] = 1.0

    sc2 = np.zeros((128, 2), dtype=np.float32)
    sc2[:, 0] = s
    sc2[:, 1] = -128.0 * s

    in_maps = []
    for c in range(N_CORES):
        # uint8 codes, transposed to [4096, 1376] then packed so partition
        # p holds, for k-tile t, row (t*128 + p): [128, 32*1376]
        wt_c = w_q[c * O_PER:(c + 1) * O_PER].T.astype(np.uint8)
        wt8_c = np.ascontiguousarray(
            wt_c.reshape(K_TILES, 128, O_PER)
            .transpose(1, 0, 2)
            .reshape(128, K_TILES * O_PER)
        )
        b = bias[c * O_PER:(c + 1) * O_PER]
        bh = b.astype(ml_dtypes.bfloat16)
        bl = (b - bh.astype(np.float32)).astype(ml_dtypes.bfloat16)
        bias_hl_c = np.ascontiguousarray(np.stack([bh, bl], axis=0))
        in_maps.append(
            {"wt8": wt8_c, "xt2": xt2, "bias_hl": bias_hl_c, "sc2": sc2}
        )
    return in_maps


def run(inputs, trace=False):
    """Run on the 8 NeuronCores. Returns (full_output, BassKernelResults)."""
    from concourse.bass_utils import run_bass_kernel_spmd

    in_maps = make_in_maps(**inputs)
    nc = _get_built()
    res = run_bass_kernel_spmd(nc, in_maps, list(range(N_CORES)), trace=trace)
    parts = [np.asarray(res.results[c]["out"]) for c in range(N_CORES)]
    full = np.concatenate(parts, axis=1)[:, :OUT_F].astype(np.float32)
    return full, res


def kernel(**inputs) -> np.ndarray:
    full, _ = run(inputs, trace=False)
    return full


# revision 10
# speedup vs baseline: 1.0676x; 1.0670x over previous
"""Trainium2 Bass kernel for nn_CachedCompressedLinear.

out[16, 11008] = x[16, 4096] @ ((w_q - 128) * scale).T + bias

Sharding: column-parallel over 8 NeuronCores; each core owns a 1376-wide
slice of out_features (8 * 1376 = 11008).

The int32 weight codes are packed to uint8 on the host (values are 0..255,
so the upper 3 bytes in HBM are zeros), cutting weight DMA 4x to
5.64 MB/core.  The weight stream is split over three DMA rings — gpsimd
SWDGE (k-tiles 0..19, fast ring, consumed first), scalar HWDGE (20..25)
and sync HWDGE (26..31) — so the slow HWDGE rings carry tiles consumed
late while the aggregate approaches the HBM ceiling.  On-device decode
uint8 -> bf16 computes (c - 128) * s directly (scale fused into the
decode, so the epilogue is a single psum_hi + psum_lo add per chunk) and
is split between DVE (cols 0:864 of each k-tile, 2x perf mode) and ACT
(cols 864:1376).  Dummy matmuls on a memset tile warm the PE's HAM clock
gate to 2.4 GHz before the first real matmul; progressive group sizes at
both ends keep the DMA->decode->MM pipeline full.  x is replicated,
pre-transposed and split into bf16 hi/lo halves for fp32-level precision;
bias is host-split into bf16 hi/lo rows and folded into the PSUM
accumulation by one K=2 matmul per chunk against a two-row one-hot block.
"""

import sys

if "/opt/trn_rl_repo" not in sys.path:
    sys.path.insert(0, "/opt/trn_rl_repo")

import numpy as np
import ml_dtypes

IN_F = 4096
OUT_F = 11008
BATCH = 16
N_CORES = 8
O_PER = 1376  # out_features per core
K_TILES = IN_F // 128  # 32
M = 48  # stationary columns: x_hi [0:16] | zeros [16:32] | x_lo [32:48]
LO = 32
# (offset, width, engine): DVE decodes [0, 864), ACT decodes [864, 1376)
CHUNKS = [(0, 512, "dve"), (512, 352, "dve"), (864, 512, "act")]
DVE_W = 864
ACT_W = 512
# k-tile counts per SWDGE transfer: small at both ends to fill/drain the
# DMA->decode->MM pipeline, large in the middle for DMA efficiency
GROUPS = [2, 2, 4, 8, 8, 4, 2, 2]
N_WARM = 12  # dummy matmuls to warm the PE clock gate
WARM_N = 256  # moving width of each warm matmul

_BUILT = None


def _build():
    """Build the (SPMD, per-core) Bass program once."""
    import concourse.bass as bass
    import concourse.tile as tile
    from concourse import bacc, mybir

    dt = mybir.dt
    alu = mybir.AluOpType
    nc = bacc.Bacc("TRN2", target_bir_lowering=False, debug=False)

    wt8 = nc.dram_tensor("wt8", [128, K_TILES * O_PER], dt.uint8,
                         kind="ExternalInput")
    xt2 = nc.dram_tensor(
        "xt2", [128, (K_TILES + 1) * M], dt.bfloat16, kind="ExternalInput"
    )
    bias_hl = nc.dram_tensor(
        "bias_hl", [2, O_PER], dt.bfloat16, kind="ExternalInput"
    )
    # col 0: s (replicated), col 1: -128*s
    sc2 = nc.dram_tensor("sc2", [128, 2], dt.float32, kind="ExternalInput")
    out = nc.dram_tensor("out", [BATCH, O_PER], dt.float32, kind="ExternalOutput")

    BIASBLK = K_TILES  # extra stationary block holding the bias one-hot
    with tile.TileContext(nc) as tc:
        with (
            tc.tile_pool(name="consts", bufs=1) as consts,
            tc.tile_pool(name="w8", bufs=4) as w8p,
            tc.tile_pool(name="wbA", bufs=3) as wbAp,
            tc.tile_pool(name="wbB", bufs=3) as wbBp,
            tc.tile_pool(name="psum", bufs=1, space=bass.MemorySpace.PSUM) as psump,
            tc.tile_pool(name="outp", bufs=1) as outp,
        ):
            # sync ring carries the small consts: scale (gates every
            # decode), then x (gates the first matmul), then bias (needed
            # at k=31).  The whole weight stream owns the fast SWDGE ring.
            sc_sb = consts.tile([128, 2], dt.float32)
            nc.sync.dma_start(sc_sb[:], sc2[:])
            x_sb = consts.tile([128, (K_TILES + 1) * M], dt.bfloat16)
            nc.sync.dma_start(x_sb[:], xt2[:])
            bias_sb = consts.tile([2, O_PER], dt.bfloat16)
            nc.sync.dma_start(bias_sb[:], bias_hl[:])

            GMAX = max(GROUPS)
            w_tiles = []
            k0 = 0
            for gi, G in enumerate(GROUPS):
                wt_t = w8p.tile([128, GMAX, O_PER], dt.uint8, tag="w8")
                nc.gpsimd.dma_start(
                    wt_t[:, 0:G, :],
                    wt8[:, k0 * O_PER:(k0 + G) * O_PER],
                )
                w_tiles.append((k0, G, wt_t))
                k0 += G

            # warm the PE clock gate with dummy matmuls on a memset tile
            warm = consts.tile([128, WARM_N], dt.bfloat16, name="warm")
            nc.vector.memset(warm[:], 1.0)
            ps_warm = psump.tile([M, WARM_N], dt.float32, name="psw", tag="psw")
            for _ in range(N_WARM):
                nc.tensor.matmul(ps_warm[:], warm[:, 0:M], warm[:],
                                 start=True, stop=True)

            psums = [
                psump.tile([M, w], dt.float32, name=f"ps{i}", tag=f"ps{i}")
                for i, (_, w, _e) in enumerate(CHUNKS)
            ]

            out_rings = [nc.sync, nc.scalar, nc.gpsimd]

            def epilogue(i, o, w):
                # TT cannot read two PSUM operands: ACT copies hi to SBUF,
                # DVE adds the lo PSUM rows
                his = outp.tile([BATCH, w], dt.float32, name=f"his{i}")
                nc.scalar.copy(his[:], psums[i][0:BATCH, :])
                comb = outp.tile([BATCH, w], dt.float32, name=f"comb{i}")
                nc.vector.tensor_add(
                    comb[:], his[:], psums[i][LO:LO + BATCH, :]
                )
                out_rings[i].dma_start(out[:][:, o:o + w], comb[:])

            for k0, G, wt_t in w_tiles:
                # decode (c-128)*s: DVE cols [0,864), ACT cols [864,1376)
                wbA = wbAp.tile([128, GMAX, DVE_W], dt.bfloat16, tag="wA")
                nc.vector.tensor_scalar(
                    wbA[:, 0:G, :], wt_t[:, 0:G, 0:DVE_W],
                    -128.0, sc_sb[:, 0:1], alu.add, alu.mult,
                )
                wbB = wbBp.tile([128, GMAX, ACT_W], dt.bfloat16, tag="wB")
                nc.scalar.activation(
                    wbB[:, 0:G, :], wt_t[:, 0:G, DVE_W:O_PER],
                    mybir.ActivationFunctionType.Identity,
                    bias=sc_sb[:, 1:2], scale=sc_sb[:, 0:1],
                )
                for t in range(G):
                    k = k0 + t
                    last = k == K_TILES - 1

                    def mv_of(i):
                        o, w, eng = CHUNKS[i]
                        if eng == "dve":
                            return wbA[:, t, o:o + w]
                        return wbB[:, t, o - DVE_W:o - DVE_W + w]

                    if not last:
                        for i in range(len(CHUNKS)):
                            nc.tensor.matmul(
                                psums[i][:, :],
                                x_sb[:, k * M:(k + 1) * M],
                                mv_of(i),
                                start=(k == 0),
                                stop=False,
                            )
                    else:
                        # close chunk-by-chunk: K=2 bias matmul + final MM
                        # per chunk; epilogue of chunk i overlaps the
                        # closing matmuls of chunk i+1
                        for i, (o, w, _e) in enumerate(CHUNKS):
                            nc.tensor.matmul(
                                psums[i][:, :],
                                x_sb[0:2, BIASBLK * M:(BIASBLK + 1) * M],
                                bias_sb[0:2, o:o + w],
                                start=False,
                                stop=False,
                            )
                            nc.tensor.matmul(
                                psums[i][:, :],
                                x_sb[:, k * M:(k + 1) * M],
                                mv_of(i),
                                start=False,
                                stop=True,
                            )
                            epilogue(i, o, w)

    nc.compile()
    return nc


def _get_built():
    global _BUILT
    if _BUILT is None:
        _BUILT = _build()
    return _BUILT


def make_in_maps(x, w_q, scale, bias):
    """Host-side shard + layout prep. Returns per-core input dicts."""
    x = np.asarray(x, dtype=np.float32)
    w_q = np.asarray(w_q, dtype=np.int32)
    scale = np.asarray(scale, dtype=np.float32)
    bias = np.asarray(bias, dtype=np.float32)
    s = float(scale.reshape(-1)[0])

    xT = np.ascontiguousarray(x.T)  # [4096, 16]
    xh = xT.astype(ml_dtypes.bfloat16)
    xl = (xT - xh.astype(np.float32)).astype(ml_dtypes.bfloat16)
    x48 = np.zeros((IN_F, M), dtype=ml_dtypes.bfloat16)  # [4096, 48]
    x48[:, :BATCH] = xh
    x48[:, LO:LO + BATCH] = xl
    # prepack to the SBUF layout [128, K_TILES*M]: partition p holds,
    # for each k-tile t, the stationary block row (t*128 + p)
    xt2 = np.zeros((128, (K_TILES + 1) * M), dtype=ml_dtypes.bfloat16)
    xt2[:, :K_TILES * M] = (
        x48.reshape(K_TILES, 128, M).transpose(1, 0, 2).reshape(128, K_TILES * M)
    )
    # bias one-hot block: partitions 0 and 1, first BATCH stationary
    # columns = 1 (K=2 matmul adds bias_hi + bias_lo into the hi rows)
    xt2[0, K_TILES * M:K_TILES * M + BATCH] = 1.0
    xt2[1, K_TILES * M:K_TILES * M + BATCH] = 1.0

    sc2 = np.zeros((128, 2), dtype=np.float32)
    sc2[:, 0] = s
    sc2[:, 1] = -128.0 * s

    in_maps = []
    for c in range(N_CORES):
        # uint8 codes, transposed to [4096, 1376] then packed so partition
        # p holds, for k-tile t, row (t*128 + p): [128, 32*1376]
        wt_c = w_q[c * O_PER:(c + 1) * O_PER].T.astype(np.uint8)
        wt8_c = np.ascontiguousarray(
            wt_c.reshape(K_TILES, 128, O_PER)
            .transpose(1, 0, 2)
            .reshape(128, K_TILES * O_PER)
        )
        b = bias[c * O_PER:(c + 1) * O_PER]
        bh = b.astype(ml_dtypes.bfloat16)
        bl = (b - bh.astype(np.float32)).astype(ml_dtypes.bfloat16)
        bias_hl_c = np.ascontiguousarray(np.stack([bh, bl], axis=0))
        in_maps.append(
            {"wt8": wt8_c, "xt2": xt2, "bias_hl": bias_hl_c, "sc2": sc2}
        )
    return in_maps


def run(inputs, trace=False):
    """Run on the 8 NeuronCores. Returns (full_output, BassKernelResults)."""
    from concourse.bass_utils import run_bass_kernel_spmd

    in_maps = make_in_maps(**inputs)
    nc = _get_built()
    res = run_bass_kernel_spmd(nc, in_maps, list(range(N_CORES)), trace=trace)
    parts = [np.asarray(res.results[c]["out"]) for c in range(N_CORES)]
    full = np.concatenate(parts, axis=1)[:, :OUT_F].astype(np.float32)
    return full, res


def kernel(**inputs) -> np.ndarray:
    full, _ = run(inputs, trace=False)
    return full


# revision 11
# speedup vs baseline: 1.1658x; 1.0920x over previous
"""Trainium2 Bass kernel for nn_CachedCompressedLinear.

out[16, 11008] = x[16, 4096] @ ((w_q - 128) * scale).T + bias

Sharding: column-parallel over 8 NeuronCores; each core owns a 1376-wide
slice of out_features (8 * 1376 = 11008).

The int32 weight codes are packed to uint8 on the host (values are 0..255,
so the upper 3 bytes in HBM are zeros), cutting weight DMA 4x to
5.64 MB/core.  The weight stream is split over three DMA rings — gpsimd
SWDGE (k-tiles 0..19, fast ring, consumed first), scalar HWDGE (20..25)
and sync HWDGE (26..31) — so the slow HWDGE rings carry tiles consumed
late while the aggregate approaches the HBM ceiling.  On-device decode
uint8 -> bf16 computes (c - 128) * s directly (scale fused into the
decode, so the epilogue is a single psum_hi + psum_lo add per chunk) and
is split between DVE (cols 0:864 of each k-tile, 2x perf mode) and ACT
(cols 864:1376).  Dummy matmuls on a memset tile warm the PE's HAM clock
gate to 2.4 GHz before the first real matmul; progressive group sizes at
both ends keep the DMA->decode->MM pipeline full.  x is replicated,
pre-transposed and split into bf16 hi/lo halves for fp32-level precision;
bias is host-split into bf16 hi/lo rows and folded into the PSUM
accumulation by one K=2 matmul per chunk against a two-row one-hot block.
"""

import sys

if "/opt/trn_rl_repo" not in sys.path:
    sys.path.insert(0, "/opt/trn_rl_repo")

import numpy as np
import ml_dtypes

IN_F = 4096
OUT_F = 11008
BATCH = 16
N_CORES = 8
O_PER = 1376  # out_features per core
K_TILES = IN_F // 128  # 32
M = 48  # stationary columns: x_hi [0:16] | zeros [16:32] | x_lo [32:48]
LO = 32
# (offset, width, engine): DVE decodes [0, 864), ACT decodes [864, 1376)
CHUNKS = [(0, 512, "dve"), (512, 352, "dve"), (864, 512, "act")]
DVE_W = 864
ACT_W = 512
# k-tile counts per SWDGE transfer: small at both ends to fill/drain the
# DMA->decode->MM pipeline, large in the middle for DMA efficiency
GROUPS = [1, 1, 2, 4, 8, 8, 4, 2, 1, 1]
N_WARM = 14  # dummy matmuls to warm the PE clock gate
WARM_N = 256  # moving width of each warm matmul

_BUILT = None


def _build():
    """Build the (SPMD, per-core) Bass program once."""
    import concourse.bass as bass
    import concourse.tile as tile
    from concourse import bacc, mybir

    dt = mybir.dt
    alu = mybir.AluOpType
    nc = bacc.Bacc("TRN2", target_bir_lowering=False, debug=False)

    wt8 = nc.dram_tensor("wt8", [128, K_TILES * O_PER], dt.uint8,
                         kind="ExternalInput")
    xt2 = nc.dram_tensor(
        "xt2", [128, (K_TILES + 1) * M], dt.bfloat16, kind="ExternalInput"
    )
    bias_hl = nc.dram_tensor(
        "bias_hl", [2, O_PER], dt.bfloat16, kind="ExternalInput"
    )
    # col 0: s (replicated), col 1: -128*s
    sc2 = nc.dram_tensor("sc2", [128, 2], dt.float32, kind="ExternalInput")
    out = nc.dram_tensor("out", [BATCH, O_PER], dt.float32, kind="ExternalOutput")

    BIASBLK = K_TILES  # extra stationary block holding the bias one-hot
    with tile.TileContext(nc) as tc:
        with (
            tc.tile_pool(name="consts", bufs=1) as consts,
            tc.tile_pool(name="w8", bufs=1) as w8p,
            tc.tile_pool(name="wbA", bufs=3) as wbAp,
            tc.tile_pool(name="wbB", bufs=3) as wbBp,
            tc.tile_pool(name="psum", bufs=1, space=bass.MemorySpace.PSUM) as psump,
            tc.tile_pool(name="outp", bufs=1) as outp,
        ):
            # sync ring carries the small consts: scale (gates every
            # decode), then x (gates the first matmul), then bias (needed
            # at k=31).  The whole weight stream owns the fast SWDGE ring.
            sc_sb = consts.tile([128, 2], dt.float32)
            nc.sync.dma_start(sc_sb[:], sc2[:])
            x_sb = consts.tile([128, (K_TILES + 1) * M], dt.bfloat16)
            nc.sync.dma_start(x_sb[:], xt2[:])
            bias_sb = consts.tile([2, O_PER], dt.bfloat16)
            nc.sync.dma_start(bias_sb[:], bias_hl[:])

            # each group gets its own exactly-sized staging buffer (the
            # buffers together hold the full 5.64 MB weight), so all
            # transfers are in flight immediately with no ring-reuse waits
            GMAX = max(GROUPS)
            w_tiles = []
            k0 = 0
            for gi, G in enumerate(GROUPS):
                wt_t = w8p.tile([128, G, O_PER], dt.uint8, tag=f"w8_{gi}")
                nc.gpsimd.dma_start(
                    wt_t[:, 0:G, :],
                    wt8[:, k0 * O_PER:(k0 + G) * O_PER],
                )
                w_tiles.append((k0, G, wt_t))
                k0 += G

            # warm the PE clock gate with dummy matmuls on a memset tile
            warm = consts.tile([128, WARM_N], dt.bfloat16, name="warm")
            nc.vector.memset(warm[:], 1.0)
            ps_warm = psump.tile([M, WARM_N], dt.float32, name="psw", tag="psw")
            for _ in range(N_WARM):
                nc.tensor.matmul(ps_warm[:], warm[:, 0:M], warm[:],
                                 start=True, stop=True)

            psums = [
                psump.tile([M, w], dt.float32, name=f"ps{i}", tag=f"ps{i}")
                for i, (_, w, _e) in enumerate(CHUNKS)
            ]

            out_rings = [nc.sync, nc.scalar, nc.gpsimd]

            def epilogue(i, o, w):
                # TT cannot read two PSUM operands: ACT copies hi to SBUF,
                # DVE adds the lo PSUM rows
                his = outp.tile([BATCH, w], dt.float32, name=f"his{i}")
                nc.scalar.copy(his[:], psums[i][0:BATCH, :])
                comb = outp.tile([BATCH, w], dt.float32, name=f"comb{i}")
                nc.vector.tensor_add(
                    comb[:], his[:], psums[i][LO:LO + BATCH, :]
                )
                out_rings[i].dma_start(out[:][:, o:o + w], comb[:])

            for k0, G, wt_t in w_tiles:
                # decode (c-128)*s: DVE cols [0,864), ACT cols [864,1376)
                wbA = wbAp.tile([128, GMAX, DVE_W], dt.bfloat16, tag="wA")
                nc.vector.tensor_scalar(
                    wbA[:, 0:G, :], wt_t[:, 0:G, 0:DVE_W],
                    -128.0, sc_sb[:, 0:1], alu.add, alu.mult,
                )
                wbB = wbBp.tile([128, GMAX, ACT_W], dt.bfloat16, tag="wB")
                nc.scalar.activation(
                    wbB[:, 0:G, :], wt_t[:, 0:G, DVE_W:O_PER],
                    mybir.ActivationFunctionType.Identity,
                    bias=sc_sb[:, 1:2], scale=sc_sb[:, 0:1],
                )
                for t in range(G):
                    k = k0 + t
                    last = k == K_TILES - 1

                    def mv_of(i):
                        o, w, eng = CHUNKS[i]
                        if eng == "dve":
                            return wbA[:, t, o:o + w]
                        return wbB[:, t, o - DVE_W:o - DVE_W + w]

                    if not last:
                        for i in range(len(CHUNKS)):
                            nc.tensor.matmul(
                                psums[i][:, :],
                                x_sb[:, k * M:(k + 1) * M],
                                mv_of(i),
                                start=(k == 0),
                                stop=False,
                            )
                    else:
                        # close chunk-by-chunk: K=2 bias matmul + final MM
                        # per chunk; epilogue of chunk i overlaps the
                        # closing matmuls of chunk i+1
                        for i, (o, w, _e) in enumerate(CHUNKS):
                            nc.tensor.matmul(
                                psums[i][:, :],
                                x_sb[0:2, BIASBLK * M:(BIASBLK + 1) * M],
                                bias_sb[0:2, o:o + w],
                                start=False,
                                stop=False,
                            )
                            nc.tensor.matmul(
                                psums[i][:, :],
                                x_sb[:, k * M:(k + 1) * M],
                                mv_of(i),
                                start=False,
                                stop=True,
                            )
                            epilogue(i, o, w)

    nc.compile()
    return nc


def _get_built():
    global _BUILT
    if _BUILT is None:
        _BUILT = _build()
    return _BUILT


def make_in_maps(x, w_q, scale, bias):
    """Host-side shard + layout prep. Returns per-core input dicts."""
    x = np.asarray(x, dtype=np.float32)
    w_q = np.asarray(w_q, dtype=np.int32)
    scale = np.asarray(scale, dtype=np.float32)
    bias = np.asarray(bias, dtype=np.float32)
    s = float(scale.reshape(-1)[0])

    xT = np.ascontiguousarray(x.T)  # [4096, 16]
    xh = xT.astype(ml_dtypes.bfloat16)
    xl = (xT - xh.astype(np.float32)).astype(ml_dtypes.bfloat16)
    x48 = np.zeros((IN_F, M), dtype=ml_dtypes.bfloat16)  # [4096, 48]
    x48[:, :BATCH] = xh
    x48[:, LO:LO + BATCH] = xl
    # prepack to the SBUF layout [128, K_TILES*M]: partition p holds,
    # for each k-tile t, the stationary block row (t*128 + p)
    xt2 = np.zeros((128, (K_TILES + 1) * M), dtype=ml_dtypes.bfloat16)
    xt2[:, :K_TILES * M] = (
        x48.reshape(K_TILES, 128, M).transpose(1, 0, 2).reshape(128, K_TILES * M)
    )
    # bias one-hot block: partitions 0 and 1, first BATCH stationary
    # columns = 1 (K=2 matmul adds bias_hi + bias_lo into the hi rows)
    xt2[0, K_TILES * M:K_TILES * M + BATCH] = 1.0
    xt2[1, K_TILES * M:K_TILES * M + BATCH] = 1.0

    sc2 = np.zeros((128, 2), dtype=np.float32)
    sc2[:, 0] = s
    sc2[:, 1] = -128.0 * s

    in_maps = []
    for c in range(N_CORES):
        # uint8 codes, transposed to [4096, 1376] then packed so partition
        # p holds, for k-tile t, row (t*128 + p): [128, 32*1376]
        wt_c = w_q[c * O_PER:(c + 1) * O_PER].T.astype(np.uint8)
        wt8_c = np.ascontiguousarray(
            wt_c.reshape(K_TILES, 128, O_PER)
            .transpose(1, 0, 2)
            .reshape(128, K_TILES * O_PER)
        )
        b = bias[c * O_PER:(c + 1) * O_PER]
        bh = b.astype(ml_dtypes.bfloat16)
        bl = (b - bh.astype(np.float32)).astype(ml_dtypes.bfloat16)
        bias_hl_c = np.ascontiguousarray(np.stack([bh, bl], axis=0))
        in_maps.append(
            {"wt8": wt8_c, "xt2": xt2, "bias_hl": bias_hl_c, "sc2": sc2}
        )
    return in_maps


def run(inputs, trace=False):
    """Run on the 8 NeuronCores. Returns (full_output, BassKernelResults)."""
    from concourse.bass_utils import run_bass_kernel_spmd

    in_maps = make_in_maps(**inputs)
    nc = _get_built()
    res = run_bass_kernel_spmd(nc, in_maps, list(range(N_CORES)), trace=trace)
    parts = [np.asarray(res.results[c]["out"]) for c in range(N_CORES)]
    full = np.concatenate(parts, axis=1)[:, :OUT_F].astype(np.float32)
    return full, res


def kernel(**inputs) -> np.ndarray:
    full, _ = run(inputs, trace=False)
    return full
